# revision 1
# baseline (speedup 1.0000x reference)
"""NLBlockND multi-cross attention block on 8 Trainium2 NeuronCores.

Per-core shard: core c handles batch b = c//2, spatial half h = c%2
(i in [h*2048, (h+1)*2048)).  All matmuls in fp32r (full-rate PE,
~1.6e-4 accuracy).  Softmax uses a constant shift of -50 which cancels
exactly in the ratio.  Conv biases b_g/b_z drop out in training-mode
BN; w_z is folded into w_g on the host (gz = (w_z@w_g) @ x).
BN batch stats are all-reduced ([128,2]) across the 8 cores.
"""
import sys
sys.path.insert(0, '/opt/trn_rl_repo')

import numpy as np

B, CIN, CI, H, W = 4, 256, 128, 64, 64
NJ = H * W              # 4096 (full spatial, j axis)
NI = NJ // 2            # 2048 per-core i positions
IC = 256                # i-chunk
NCH = NI // IC          # 8 chunks
JBS = NJ // 128         # 32 j-blocks
JGROUPS = [4] * 8   # j-blocks per exp group (PSUM: 2 banks)
SHIFT = -50.0
BN_EPS = 1e-5
NTOT = float(B * NJ)    # BN count per channel

_CACHE = {}


def _build(repeat=1, hoist=True, unroll=1):
    import concourse.bacc as bacc
    import concourse.mybir as mybir
    from concourse import tile

    dt = mybir.dt
    AF = mybir.ActivationFunctionType

    nc = bacc.Bacc("TRN2", target_bir_lowering=False, debug=False, num_devices=8)

    xt = nc.dram_tensor("xt", [CIN, NJ], dt.float32r, kind="ExternalInput").ap()
    xo = nc.dram_tensor("xo", [CIN, NI], dt.float32r, kind="ExternalInput").ap()
    wtT = nc.dram_tensor("wtT", [CIN, CI], dt.float32r, kind="ExternalInput").ap()
    wpT = nc.dram_tensor("wpT", [CIN, CI], dt.float32r, kind="ExternalInput").ap()
    wzgT = nc.dram_tensor("wzgT", [CIN, CI], dt.float32r, kind="ExternalInput").ap()
    # consts columns: 0 b_theta, 1 b_phi, 2 gamma, 3 beta, 4 SHIFT, 5 eps, 6 1/NTOT
    consts = nc.dram_tensor("consts", [CI, 8], dt.float32, kind="ExternalInput").ap()
    zout_d = nc.dram_tensor("z", [CI, NI], dt.float32, kind="ExternalOutput").ap()

    cc_in = nc.dram_tensor("cc_in", [CI, 2], dt.float32)
    cc_out = nc.dram_tensor("cc_out", [8 * CI, 2], dt.float32, addr_space="Shared")

    with tile.TileContext(nc) as tc:
        with tc.tile_pool(name="big", bufs=1) as bigp, \
             tc.tile_pool(name="exp", bufs=2) as expp, \
             tc.tile_pool(name="sm", bufs=2) as smp, \
             tc.tile_pool(name="S", bufs=2, space="PSUM") as Sp, \
             tc.tile_pool(name="zp", bufs=2, space="PSUM") as zpp, \
             tc.tile_pool(name="rsp", bufs=1, space="PSUM") as rsp, \
             tc.tile_pool(name="gzp", bufs=1, space="PSUM") as gzp, \
             tc.tile_pool(name="ld", bufs=1) as ldp:

          for _rep in range(unroll):
              # ---- DRAM loads, ordered for earliest compute start ----
              wtT_r = [ldp.tile([128, CI], dt.float32r, tag=f"wt{c}", name=f"wtT_r{c}") for c in range(2)]
              wpT_r = [ldp.tile([128, CI], dt.float32r, tag=f"wp{c}", name=f"wpT_r{c}") for c in range(2)]
              wzgT_r = [ldp.tile([128, CI], dt.float32r, tag=f"wz{c}", name=f"wzgT_r{c}") for c in range(2)]
              for c in range(2):
                  nc.sync.dma_start(wtT_r[c][:], wtT[c * 128:(c + 1) * 128, :])
                  nc.sync.dma_start(wpT_r[c][:], wpT[c * 128:(c + 1) * 128, :])
                  nc.sync.dma_start(wzgT_r[c][:], wzgT[c * 128:(c + 1) * 128, :])
              cst = bigp.tile([CI, 8], dt.float32, tag="cst")
              nc.sync.dma_start(cst[:], consts[:])
              ones_f = bigp.tile([128, 128], dt.float32, tag="ones_f")
              nc.gpsimd.memset(ones_f[:], 1.0)
              ones_r = bigp.tile([128, 128], dt.float32r, tag="ones_r")
              nc.vector.tensor_copy(ones_r[:], ones_f[:])

              xo_r = [ldp.tile([128, NI], dt.float32r, tag=f"xo{c}", name=f"xo_r{c}") for c in range(2)]
              for c in range(2):
                  nc.sync.dma_start(xo_r[c][:, 0:512], xo[c * 128:(c + 1) * 128, 0:512])
              xt_r = [ldp.tile([128, NJ], dt.float32r, tag=f"xt{c}", name=f"xt_r{c}") for c in range(2)]
              for jc in range(NJ // 512):
                  cs = slice(jc * 512, (jc + 1) * 512)
                  for c in range(2):
                      nc.sync.dma_start(xt_r[c][:, cs], xt[c * 128:(c + 1) * 128, cs])
              for c in range(2):
                  nc.sync.dma_start(xo_r[c][:, 512:NI], xo[c * 128:(c + 1) * 128, 512:NI])

              # ---- projections ----
              phi = bigp.tile([128, NI], dt.float32r, tag="phi")
              ps = Sp.tile([128, 512], dt.float32, tag="S", name="ps_phi0")
              for c in range(2):
                  nc.tensor.matmul(ps[:], wpT_r[c][:], xo_r[c][:, 0:512],
                                   start=(c == 0), stop=(c == 1))
              nc.vector.tensor_scalar_add(phi[:, 0:512], ps[:], cst[:, 1:2])

              theta = bigp.tile([128, NJ], dt.float32r, tag="theta")
              for jc in range(NJ // 512):
                  ps = Sp.tile([128, 512], dt.float32, tag="S", name="ps_th")
                  for c in range(2):
                      nc.tensor.matmul(ps[:], wtT_r[c][:],
                                       xt_r[c][:, jc * 512:(jc + 1) * 512],
                                       start=(c == 0), stop=(c == 1))
                  nc.vector.tensor_scalar_add(theta[:, jc * 512:(jc + 1) * 512],
                                              ps[:], cst[:, 0:1])

              z_sb = bigp.tile([128, NI], dt.float32, tag="z_sb")
              gzT = bigp.tile([128, NJ], dt.float32r, tag="gzT")

              e_chunks = {}

              def emit_scores_group(k, g0, gn, interleave=None):
                  pk = slice(k * IC, (k + 1) * IC)
                  e3 = e_chunks[k][:].rearrange("p (i j) -> p j i", j=JBS)
                  S_ps = Sp.tile([128, 1024], dt.float32, tag="S", name="S_ps")
                  for jj in range(gn):
                      jb = g0 + jj
                      nc.tensor.matmul(S_ps[:, jj * IC:(jj + 1) * IC],
                                       theta[:, jb * 128:(jb + 1) * 128],
                                       phi[:, pk], start=True, stop=True)
                      if interleave is not None:
                          interleave(jb)
                  nc.scalar.activation(
                      e3[:, g0:g0 + gn, :],
                      S_ps[:].rearrange("p (j i) -> p j i", i=IC)[:, 0:gn, :],
                      AF.Exp, bias=cst[:, 4:5])

              def emit_gzT_block(jb):
                  ps = gzp.tile([128, 128], dt.float32, tag="gz", name="ps_gz")
                  for c in range(2):
                      nc.tensor.matmul(ps[:], xt_r[c][:, jb * 128:(jb + 1) * 128],
                                       wzgT_r[c][:], start=(c == 0), stop=(c == 1))
                  nc.vector.tensor_copy(gzT[:, jb * 128:(jb + 1) * 128], ps[:])

              # hoisted chunk 0: scores interleaved with gzT projection +
              # remaining phi chunks
              def chunk0_phi_chunk(jb):
                  # phi chunks 1..3 (cols 512..2048)
                  ps = Sp.tile([128, 512], dt.float32, tag="S", name="ps_phi")
                  cs = slice((jb + 1) * 512, (jb + 2) * 512)
                  for c in range(2):
                      nc.tensor.matmul(ps[:], wpT_r[c][:], xo_r[c][:, cs],
                                       start=(c == 0), stop=(c == 1))
                  nc.vector.tensor_scalar_add(phi[:, cs], ps[:], cst[:, 1:2])

              def chunk0_interleave(jb):
                  emit_gzT_block(jb)
                  if jb < 3:
                      chunk0_phi_chunk(jb)

              if hoist:
                  e_chunks[0] = expp.tile([128, IC * JBS], dt.float32r,
                                          tag="e", name="e0")
                  g0 = 0
                  for g, gn in enumerate(JGROUPS):
                      emit_scores_group(0, g0, gn, interleave=chunk0_interleave)
                      g0 += gn
              else:
                  for jb in range(JBS):
                      emit_gzT_block(jb)
                      if jb < 3:
                          chunk0_phi_chunk(jb)

              stat = bigp.tile([128, 2], dt.float32, tag="stat")

              def main_chunks(iv=None):
                for k in range(NCH):
                  pk = slice(k * IC, (k + 1) * IC)
                  if k > 0 or not hoist:
                      e_chunks[k] = expp.tile([128, IC * JBS], dt.float32r,
                                              tag="e", name="e")
                      g0 = 0
                      for g, gn in enumerate(JGROUPS):
                          emit_scores_group(k, g0, gn)
                          g0 += gn
                  e_chunk = e_chunks[k]
                  e3 = e_chunk[:].rearrange("p (i j) -> p j i", j=JBS)

                  # z' accumulation over all 32 j-blocks
                  zpart = zpp.tile([128, IC], dt.float32, tag="zp")
                  for jb in range(JBS):
                      nc.tensor.matmul(zpart[:], gzT[:, jb * 128:(jb + 1) * 128],
                                       e3[:, jb, :], start=(jb == 0),
                                       stop=(jb == JBS - 1))

                  # softmax sums over jb: gpsimd tree (jb<12), DVE reduce (jb>=12)
                  ei = e_chunk[:].rearrange("p (i j) -> p i j", j=JBS).bitcast(dt.float32)
                  t8 = smp.tile([128, IC * 8], dt.float32, tag="t8", bufs=1)
                  t8v = t8[:].rearrange("p (i j) -> p i j", j=8)
                  nc.gpsimd.tensor_add(t8v[:], ei[:, :, 0:8], ei[:, :, 8:16])
                  t4 = smp.tile([128, IC * 4], dt.float32, tag="t4", bufs=1)
                  t4v = t4[:].rearrange("p (i j) -> p i j", j=4)
                  nc.gpsimd.tensor_add(t4v[:], t8v[:, :, 0:4], t8v[:, :, 4:8])
                  t2 = smp.tile([128, IC * 2], dt.float32, tag="t2", bufs=1)
                  t2v = t2[:].rearrange("p (i j) -> p i j", j=2)
                  nc.gpsimd.tensor_add(t2v[:], t4v[:, :, 0:2], t4v[:, :, 2:4])
                  s_b = smp.tile([128, IC], dt.float32, tag="sb")
                  nc.gpsimd.tensor_add(s_b[:], t2v[:, :, 0], t2v[:, :, 1])
                  s_a = smp.tile([128, IC], dt.float32, tag="sa")
                  nc.vector.reduce_sum(s_a[:], ei[:, :, 16:32],
                                       axis=mybir.AxisListType.X)
                  s_part = smp.tile([128, IC], dt.float32r, tag="sp")
                  with nc.allow_low_precision(reason="f32r out, fp32 internal"):
                      nc.vector.tensor_add(s_part[:], s_a[:], s_b[:])
                  rs = rsp.tile([128, IC], dt.float32, tag="rs")
                  nc.tensor.matmul(rs[:], ones_r[:], s_part[:], start=True, stop=True)
                  rrs = smp.tile([128, IC], dt.float32, tag="rrs")
                  nc.vector.reciprocal(rrs[:], rs[:])
                  nc.vector.tensor_mul(z_sb[:, pk], zpart[:], rrs[:])

                  # per-chunk BN stat partials
                  sq = smp.tile([128, IC], dt.float32, tag="sq")
                  nc.vector.tensor_mul(sq[:], z_sb[:, pk], z_sb[:, pk])
                  s1c = smp.tile([128, 1], dt.float32, tag="s1c")
                  nc.vector.reduce_sum(s1c[:], z_sb[:, pk], axis=mybir.AxisListType.X)
                  s2c = smp.tile([128, 1], dt.float32, tag="s2c")
                  nc.vector.reduce_sum(s2c[:], sq[:], axis=mybir.AxisListType.X)
                  if k == 0:
                      nc.vector.tensor_copy(stat[:, 0:1], s1c[:])
                      nc.vector.tensor_copy(stat[:, 1:2], s2c[:])
                  else:
                      nc.vector.tensor_add(stat[:, 0:1], stat[:, 0:1], s1c[:])
                      nc.vector.tensor_add(stat[:, 1:2], stat[:, 1:2], s2c[:])

              if repeat == 1:
                  main_chunks()
              else:
                  with tc.For_i(0, repeat, 1) as iv:
                      main_chunks(iv)

              # ---- BN stats AllGather + local reduce ----
              nc.sync.dma_start(cc_in.ap()[:], stat[:])
              nc.gpsimd.collective_compute(
                  "AllGather", mybir.AluOpType.bypass,
                  replica_groups=[list(range(8))],
                  ins=[cc_in.ap().opt()], outs=[cc_out.ap().opt()])
              stat_ag = bigp.tile([128, 16], dt.float32, tag="stat_ag")
              # cc_out is [8*128, 2] (shards along axis 0); gather to [p, (s c)]
              ag_view = cc_out.ap()[:].rearrange("(s p) c -> p s c", s=8)
              nc.sync.dma_start(stat_ag[:].rearrange("p (s c) -> p s c", c=2), ag_view)
              agv = stat_ag[:].rearrange("p (s c) -> p s c", c=2)
              ag4 = bigp.tile([128, 8], dt.float32, tag="ag4")
              ag4v = ag4[:].rearrange("p (s c) -> p s c", c=2)
              nc.vector.tensor_add(ag4v[:], agv[:, 0:4, :], agv[:, 4:8, :])
              ag2 = bigp.tile([128, 4], dt.float32, tag="ag2")
              ag2v = ag2[:].rearrange("p (s c) -> p s c", c=2)
              nc.vector.tensor_add(ag2v[:], ag4v[:, 0:2, :], ag4v[:, 2:4, :])
              stat_all = bigp.tile([128, 2], dt.float32, tag="stat_all")
              nc.vector.tensor_add(stat_all[:], ag2v[:, 0, :], ag2v[:, 1, :])

              # mean = S1/NTOT ; ex2 = S2/NTOT ; var = ex2 - mean^2
              me = bigp.tile([128, 2], dt.float32, tag="me")
              nc.vector.tensor_scalar_mul(me[:], stat_all[:], cst[:, 6:7])
              mean = me[:, 0:1]
              msq = bigp.tile([128, 1], dt.float32, tag="msq")
              nc.vector.tensor_mul(msq[:], mean, mean)
              var = bigp.tile([128, 1], dt.float32, tag="var")
              nc.vector.tensor_sub(var[:], me[:, 1:2], msq[:])
              std = bigp.tile([128, 1], dt.float32, tag="std")
              nc.scalar.activation(std[:], var[:], AF.Sqrt, bias=cst[:, 5:6])
              rstd = bigp.tile([128, 1], dt.float32, tag="rstd")
              nc.vector.reciprocal(rstd[:], std[:])
              scale = bigp.tile([128, 1], dt.float32, tag="scale")
              nc.vector.tensor_mul(scale[:], rstd[:], cst[:, 2:3])
              mscale = bigp.tile([128, 1], dt.float32, tag="mscale")
              nc.vector.tensor_mul(mscale[:], mean, scale[:])
              bias2 = bigp.tile([128, 1], dt.float32, tag="bias2")
              nc.vector.tensor_sub(bias2[:], cst[:, 3:4], mscale[:])

              # apply + store, split for ACT/DMA overlap
              zfin = bigp.tile([128, NI], dt.float32, tag="zfin")
              for h in range(4):
                  cs = slice(h * (NI // 4), (h + 1) * (NI // 4))
                  nc.scalar.activation(zfin[:, cs], z_sb[:, cs], AF.Identity,
                                       bias=bias2[:], scale=scale[:])
                  nc.sync.dma_start(zout_d[:, cs], zfin[:, cs])

    nc.compile()

    return nc


def _prep_in_maps(inputs):
    xt_full = np.ascontiguousarray(
        inputs['x_thisBranch'].reshape(B, CIN, NJ).astype(np.float32))
    xo_full = np.ascontiguousarray(
        inputs['x_otherBranch'].reshape(B, CIN, NJ).astype(np.float32))
    wtT = np.ascontiguousarray(inputs['w_theta'].T.astype(np.float32))
    wpT = np.ascontiguousarray(inputs['w_phi'].T.astype(np.float32))
    w_zg = (inputs['w_z'].astype(np.float64) @ inputs['w_g'].astype(np.float64))
    wzgT = np.ascontiguousarray(w_zg.T.astype(np.float32))
    consts = np.zeros((CI, 8), np.float32)
    consts[:, 0] = inputs['b_theta']
    consts[:, 1] = inputs['b_phi']
    consts[:, 2] = inputs['bn_gamma']
    consts[:, 3] = inputs['bn_beta']
    consts[:, 4] = SHIFT
    consts[:, 5] = BN_EPS
    consts[:, 6] = 1.0 / NTOT
    in_maps = []
    for c in range(8):
        b, h = c // 2, c % 2
        in_maps.append({
            "xt": xt_full[b],
            "xo": np.ascontiguousarray(xo_full[b][:, h * NI:(h + 1) * NI]),
            "wtT": wtT, "wpT": wpT, "wzgT": wzgT, "consts": consts,
        })
    return in_maps


def kernel(**inputs):
    from concourse.bass_utils import run_bass_kernel_spmd
    if "nc" not in _CACHE:
        _CACHE["nc"] = _build()
    nc = _CACHE["nc"]
    in_maps = _prep_in_maps(inputs)
    res = run_bass_kernel_spmd(nc, in_maps, list(range(8)))
    out = np.empty((B, CI, NJ), np.float32)
    for c in range(8):
        b, h = c // 2, c % 2
        out[b][:, h * NI:(h + 1) * NI] = res.results[c]["z"]
    return out.reshape(B, CI, H, W)


if __name__ == "__main__":
    inputs = np.load('/tmp/ref_inputs.npy', allow_pickle=True).item()
    ref = np.load('/tmp/ref_output.npy')
    got = kernel(**inputs)
    err = np.abs(got - ref)
    denom = np.abs(ref).max()
    print(f"abs max err: {err.max():.4e}  (ref absmax {denom:.3f})")
    print(f"Relative error: {err.max() / denom:.4e}")



# revision 4
# speedup vs baseline: 2.6380x; 2.6380x over previous
"""NLBlockND multi-cross attention block on 8 Trainium2 NeuronCores.

Per-core shard: core c handles batch b = c//2, spatial half h = c%2
(i in [h*2048, (h+1)*2048)).  bf16 datapath: all matmul operands bf16
(full PE rate at any moving width + FWL fast weight loads), PSUM/stats
fp32.  Softmax uses a constant shift of -50 which cancels in the ratio;
the theta conv bias is constant per softmax row so it cancels exactly
and is dropped; phi's bias is folded into the PSUM drain.  b_g/b_z drop
out in training-mode BN; w_z is folded into w_g on the host.  Softmax
j-reduction: 24 j-blocks tree-reduced on DVE (packed bf16, 2x mode),
8 on GPSIMD.  BN rstd = exp(-0.5*ln(var+eps)) so every activation stays
in one ACT table set (no reloads).  Cross-rep tiles (theta/phi/gzT/
z_sb/stat) are double-buffered so one rep's BN-stats AllGather + apply
tail overlaps the next rep's compute.  BN batch stats are all-gathered
([128,2]) across the 8 cores.
"""
import sys
sys.path.insert(0, '/opt/trn_rl_repo')

import numpy as np

B, CIN, CI, H, W = 4, 256, 128, 64, 64
NJ = H * W              # 4096 (full spatial, j axis)
NI = NJ // 2            # 2048 per-core i positions
IC = 256                # i-chunk
NCH = NI // IC          # 8 chunks
JBS = NJ // 128         # 32 j-blocks
JGROUPS = [4] * 8       # j-blocks per exp group (PSUM: 2 banks)
MD = 24                 # j-blocks tree-reduced on DVE; JBS-MD on GPSIMD
SHIFT = -50.0
BN_EPS = 1e-5
NTOT = float(B * NJ)    # BN count per channel

_CACHE = {}


def _build(unroll=1):
    import concourse.bacc as bacc
    import concourse.mybir as mybir
    from concourse import tile

    dt = mybir.dt
    AF = mybir.ActivationFunctionType
    ALU = mybir.AluOpType

    nc = bacc.Bacc("TRN2", target_bir_lowering=False, debug=False, num_devices=8)

    xtb = nc.dram_tensor("xtb", [CIN, NJ], dt.bfloat16, kind="ExternalInput").ap()
    xob = nc.dram_tensor("xob", [CIN, NI], dt.bfloat16, kind="ExternalInput").ap()
    wtT_d = nc.dram_tensor("wtT", [CIN, CI], dt.bfloat16, kind="ExternalInput").ap()
    wpT_d = nc.dram_tensor("wpT", [CIN, CI], dt.bfloat16, kind="ExternalInput").ap()
    wzgT_d = nc.dram_tensor("wzgT", [CIN, CI], dt.bfloat16, kind="ExternalInput").ap()
    # consts columns: 0 b_phi, 1 gamma, 2 beta, 3 SHIFT, 4 eps, 5 1/NTOT
    consts = nc.dram_tensor("consts", [CI, 8], dt.float32, kind="ExternalInput").ap()
    zout_d = nc.dram_tensor("z", [CI, NI], dt.float32, kind="ExternalOutput").ap()

    cc_in = nc.dram_tensor("cc_in", [CI, 2], dt.float32)
    cc_out = nc.dram_tensor("cc_out", [8 * CI, 2], dt.float32, addr_space="Shared")

    with tile.TileContext(nc) as tc, \
         nc.allow_low_precision(reason="bf16 datapath, fp32 accumulation"):
        with tc.tile_pool(name="ld", bufs=1) as ldp, \
             tc.tile_pool(name="big", bufs=1) as bigp, \
             tc.tile_pool(name="db", bufs=2) as dbp, \
             tc.tile_pool(name="exp", bufs=2) as expp, \
             tc.tile_pool(name="sm", bufs=2) as smp, \
             tc.tile_pool(name="S", bufs=2, space="PSUM") as Sp, \
             tc.tile_pool(name="zp", bufs=2, space="PSUM") as zpp, \
             tc.tile_pool(name="rsp", bufs=1, space="PSUM") as rsp, \
             tc.tile_pool(name="gzp", bufs=1, space="PSUM") as gzp:

          for _rep in range(unroll):
              # ---- DRAM loads, ordered for earliest compute start ----
              wtT_r = [ldp.tile([128, CI], dt.bfloat16, tag=f"wt{c}", name=f"wtT_r{c}") for c in range(2)]
              wpT_r = [ldp.tile([128, CI], dt.bfloat16, tag=f"wp{c}", name=f"wpT_r{c}") for c in range(2)]
              wzgT_r = [ldp.tile([128, CI], dt.bfloat16, tag=f"wz{c}", name=f"wzgT_r{c}") for c in range(2)]
              for c in range(2):
                  nc.sync.dma_start(wtT_r[c][:], wtT_d[c * 128:(c + 1) * 128, :])
                  nc.sync.dma_start(wpT_r[c][:], wpT_d[c * 128:(c + 1) * 128, :])
                  nc.sync.dma_start(wzgT_r[c][:], wzgT_d[c * 128:(c + 1) * 128, :])
              cst = bigp.tile([CI, 8], dt.float32, tag="cst")
              nc.sync.dma_start(cst[:], consts[:])
              ones_b = bigp.tile([128, 128], dt.bfloat16, tag="ones_b")
              nc.gpsimd.memset(ones_b[:], 1.0)

              xo_r = [ldp.tile([128, NI], dt.bfloat16, tag=f"xo{c}", name=f"xo_r{c}") for c in range(2)]
              for c in range(2):
                  nc.sync.dma_start(xo_r[c][:, 0:512], xob[c * 128:(c + 1) * 128, 0:512])
              xt_r = [ldp.tile([128, NJ], dt.bfloat16, tag=f"xt{c}", name=f"xt_r{c}") for c in range(2)]
              for jc in range(NJ // 1024):
                  cs = slice(jc * 1024, (jc + 1) * 1024)
                  for c in range(2):
                      nc.sync.dma_start(xt_r[c][:, cs], xtb[c * 128:(c + 1) * 128, cs])
              for c in range(2):
                  nc.sync.dma_start(xo_r[c][:, 512:NI], xob[c * 128:(c + 1) * 128, 512:NI])

              # ---- projections ----
              phi = dbp.tile([128, NI], dt.bfloat16, tag="phi")
              ps = Sp.tile([128, 512], dt.float32, tag="S", name="ps_phi0")
              for c in range(2):
                  nc.tensor.matmul(ps[:], wpT_r[c][:], xo_r[c][:, 0:512],
                                   start=(c == 0), stop=(c == 1))
              nc.vector.tensor_scalar_add(phi[:, 0:512], ps[:], cst[:, 0:1])

              theta = dbp.tile([128, NJ], dt.bfloat16, tag="theta")
              for jc in range(NJ // 512):
                  ps = Sp.tile([128, 512], dt.float32, tag="S", name="ps_th")
                  for c in range(2):
                      nc.tensor.matmul(ps[:], wtT_r[c][:],
                                       xt_r[c][:, jc * 512:(jc + 1) * 512],
                                       start=(c == 0), stop=(c == 1))
                  # theta bias is constant per softmax row -> cancels; plain copy
                  nc.vector.tensor_copy(theta[:, jc * 512:(jc + 1) * 512], ps[:])

              z_sb = dbp.tile([128, NI], dt.float32, tag="z_sb")
              gzT = dbp.tile([128, NJ], dt.bfloat16, tag="gzT")

              e_chunks = {}

              def emit_scores_group(k, g0, gn, interleave=None):
                  pk = slice(k * IC, (k + 1) * IC)
                  e3 = e_chunks[k][:].rearrange("p (i j) -> p j i", j=JBS)
                  S_ps = Sp.tile([128, 1024], dt.float32, tag="S", name="S_ps")
                  for jj in range(gn):
                      jb = g0 + jj
                      nc.tensor.matmul(S_ps[:, jj * IC:(jj + 1) * IC],
                                       theta[:, jb * 128:(jb + 1) * 128],
                                       phi[:, pk], start=True, stop=True)
                      if interleave is not None:
                          interleave(jb)
                  nc.scalar.activation(
                      e3[:, g0:g0 + gn, :],
                      S_ps[:].rearrange("p (j i) -> p j i", i=IC)[:, 0:gn, :],
                      AF.Exp, bias=cst[:, 3:4])

              def emit_gzT_block(jb):
                  ps = gzp.tile([128, 128], dt.float32, tag="gz", name="ps_gz")
                  for c in range(2):
                      nc.tensor.matmul(ps[:], xt_r[c][:, jb * 128:(jb + 1) * 128],
                                       wzgT_r[c][:], start=(c == 0), stop=(c == 1))
                  nc.vector.tensor_copy(gzT[:, jb * 128:(jb + 1) * 128], ps[:])

              # hoisted chunk 0: scores interleaved with gzT projection +
              # remaining phi chunks
              def chunk0_phi_chunk(jb):
                  # phi chunks 1..3 (cols 512..2048)
                  ps = Sp.tile([128, 512], dt.float32, tag="S", name="ps_phi")
                  cs = slice((jb + 1) * 512, (jb + 2) * 512)
                  for c in range(2):
                      nc.tensor.matmul(ps[:], wpT_r[c][:], xo_r[c][:, cs],
                                       start=(c == 0), stop=(c == 1))
                  nc.vector.tensor_scalar_add(phi[:, cs], ps[:], cst[:, 0:1])

              def chunk0_interleave(jb):
                  emit_gzT_block(jb)
                  if jb < 3:
                      chunk0_phi_chunk(jb)

              e_chunks[0] = expp.tile([128, IC * JBS], dt.bfloat16,
                                      tag="e", name="e0")
              g0 = 0
              for g, gn in enumerate(JGROUPS):
                  emit_scores_group(0, g0, gn, interleave=chunk0_interleave)
                  g0 += gn

              stat = dbp.tile([128, 2], dt.float32, tag="stat")

              for k in range(NCH):
                  pk = slice(k * IC, (k + 1) * IC)
                  if k > 0:
                      e_chunks[k] = expp.tile([128, IC * JBS], dt.bfloat16,
                                              tag="e", name="e")
                      g0 = 0
                      for g, gn in enumerate(JGROUPS):
                          emit_scores_group(k, g0, gn)
                          g0 += gn
                  e_chunk = e_chunks[k]
                  e3 = e_chunk[:].rearrange("p (i j) -> p j i", j=JBS)

                  # z' accumulation over all 32 j-blocks
                  zpart = zpp.tile([128, IC], dt.float32, tag="zp")
                  for jb in range(JBS):
                      nc.tensor.matmul(zpart[:], gzT[:, jb * 128:(jb + 1) * 128],
                                       e3[:, jb, :], start=(jb == 0),
                                       stop=(jb == JBS - 1))

                  # softmax sums over jb: DVE packed-bf16 tree (jb<24),
                  # GPSIMD tree (jb>=24)
                  ei = e_chunk[:].rearrange("p (i j) -> p i j", j=JBS)
                  t12 = smp.tile([128, IC * 12], dt.bfloat16, tag="t12", bufs=1)
                  t12v = t12[:].rearrange("p (i j) -> p i j", j=12)
                  nc.vector.tensor_add(t12v[:], ei[:, :, 0:12], ei[:, :, 12:24])
                  t6 = smp.tile([128, IC * 6], dt.bfloat16, tag="t6", bufs=1)
                  t6v = t6[:].rearrange("p (i j) -> p i j", j=6)
                  nc.vector.tensor_add(t6v[:], t12v[:, :, 0:6], t12v[:, :, 6:12])
                  t3 = smp.tile([128, IC * 3], dt.bfloat16, tag="t3", bufs=1)
                  t3v = t3[:].rearrange("p (i j) -> p i j", j=3)
                  nc.vector.tensor_add(t3v[:], t6v[:, :, 0:3], t6v[:, :, 3:6])
                  tx = smp.tile([128, IC], dt.float32, tag="tx", bufs=1)
                  nc.vector.tensor_add(tx[:], t3v[:, :, 0], t3v[:, :, 1])
                  s_bD = smp.tile([128, IC], dt.float32, tag="sbD", bufs=1)
                  nc.vector.tensor_add(s_bD[:], tx[:], t3v[:, :, 2])

                  u4 = smp.tile([128, IC * 4], dt.bfloat16, tag="u4", bufs=1)
                  u4v = u4[:].rearrange("p (i j) -> p i j", j=4)
                  nc.gpsimd.tensor_add(u4v[:], ei[:, :, 24:28], ei[:, :, 28:32])
                  u2 = smp.tile([128, IC * 2], dt.bfloat16, tag="u2", bufs=1)
                  u2v = u2[:].rearrange("p (i j) -> p i j", j=2)
                  nc.gpsimd.tensor_add(u2v[:], u4v[:, :, 0:2], u4v[:, :, 2:4])
                  s_bP = smp.tile([128, IC], dt.float32, tag="sbP", bufs=1)
                  nc.gpsimd.tensor_add(s_bP[:], u2v[:, :, 0], u2v[:, :, 1])

                  s_part = smp.tile([128, IC], dt.bfloat16, tag="sp")
                  nc.vector.tensor_add(s_part[:], s_bD[:], s_bP[:])
                  rs = rsp.tile([128, IC], dt.float32, tag="rs")
                  nc.tensor.matmul(rs[:], ones_b[:], s_part[:], start=True, stop=True)
                  rrs = smp.tile([128, IC], dt.float32, tag="rrs")
                  nc.vector.reciprocal(rrs[:], rs[:])

                  # normalize + BN stat partials (fused reduces)
                  s1c = smp.tile([128, 1], dt.float32, tag="s1c")
                  s2c = smp.tile([128, 1], dt.float32, tag="s2c")
                  nc.vector.tensor_tensor_reduce(
                      z_sb[:, pk], zpart[:], rrs[:], 1.0, 0.0,
                      ALU.mult, ALU.add, s1c[:])
                  sq = smp.tile([128, IC], dt.float32, tag="sq")
                  nc.vector.tensor_tensor_reduce(
                      sq[:], z_sb[:, pk], z_sb[:, pk], 1.0, 0.0,
                      ALU.mult, ALU.add, s2c[:])
                  if k == 0:
                      nc.vector.tensor_copy(stat[:, 0:1], s1c[:])
                      nc.vector.tensor_copy(stat[:, 1:2], s2c[:])
                  else:
                      nc.vector.tensor_add(stat[:, 0:1], stat[:, 0:1], s1c[:])
                      nc.vector.tensor_add(stat[:, 1:2], stat[:, 1:2], s2c[:])

              # ---- BN stats AllGather + local reduce ----
              nc.sync.dma_start(cc_in.ap()[:], stat[:])
              nc.gpsimd.collective_compute(
                  "AllGather", mybir.AluOpType.bypass,
                  replica_groups=[list(range(8))],
                  ins=[cc_in.ap().opt()], outs=[cc_out.ap().opt()])
              stat_ag = dbp.tile([128, 16], dt.float32, tag="stat_ag")
              # cc_out is [8*128, 2] (shards along axis 0); gather to [p, (s c)]
              ag_view = cc_out.ap()[:].rearrange("(s p) c -> p s c", s=8)
              nc.sync.dma_start(stat_ag[:].rearrange("p (s c) -> p s c", c=2), ag_view)
              agv = stat_ag[:].rearrange("p (s c) -> p s c", c=2)
              ag4 = dbp.tile([128, 8], dt.float32, tag="ag4")
              ag4v = ag4[:].rearrange("p (s c) -> p s c", c=2)
              nc.vector.tensor_add(ag4v[:], agv[:, 0:4, :], agv[:, 4:8, :])
              ag2 = dbp.tile([128, 4], dt.float32, tag="ag2")
              ag2v = ag2[:].rearrange("p (s c) -> p s c", c=2)
              nc.vector.tensor_add(ag2v[:], ag4v[:, 0:2, :], ag4v[:, 2:4, :])
              stat_all = dbp.tile([128, 2], dt.float32, tag="stat_all")
              nc.vector.tensor_add(stat_all[:], ag2v[:, 0, :], ag2v[:, 1, :])

              # mean = S1/NTOT ; ex2 = S2/NTOT ; var = ex2 - mean^2
              # rstd = exp(-0.5*ln(var+eps)) -- stays in the exp ACT table set
              me = dbp.tile([128, 2], dt.float32, tag="me")
              nc.vector.tensor_scalar_mul(me[:], stat_all[:], cst[:, 5:6])
              mean = me[:, 0:1]
              msq = dbp.tile([128, 1], dt.float32, tag="msq")
              nc.vector.tensor_mul(msq[:], mean, mean)
              var = dbp.tile([128, 1], dt.float32, tag="var")
              nc.vector.tensor_sub(var[:], me[:, 1:2], msq[:])
              lnv = dbp.tile([128, 1], dt.float32, tag="lnv")
              nc.scalar.activation(lnv[:], var[:], AF.Ln, bias=cst[:, 4:5])
              rstd = dbp.tile([128, 1], dt.float32, tag="rstd")
              nc.scalar.activation(rstd[:], lnv[:], AF.Exp, scale=-0.5)
              scale_t = dbp.tile([128, 1], dt.float32, tag="scale")
              nc.vector.tensor_mul(scale_t[:], rstd[:], cst[:, 1:2])
              mscale = dbp.tile([128, 1], dt.float32, tag="mscale")
              nc.vector.tensor_mul(mscale[:], mean, scale_t[:])
              bias2 = dbp.tile([128, 1], dt.float32, tag="bias2")
              nc.vector.tensor_sub(bias2[:], cst[:, 2:3], mscale[:])

              # apply (DVE affine) + store, split for DVE/DMA overlap
              zfin = dbp.tile([128, NI], dt.float32, tag="zfin")
              for h in range(4):
                  cs = slice(h * (NI // 4), (h + 1) * (NI // 4))
                  nc.vector.tensor_scalar(zfin[:, cs], z_sb[:, cs],
                                          scale_t[:], bias2[:],
                                          ALU.mult, ALU.add)
                  nc.sync.dma_start(zout_d[:, cs], zfin[:, cs])

    nc.compile()

    return nc


def _prep_in_maps(inputs):
    import ml_dtypes
    bf16 = ml_dtypes.bfloat16
    xt_full = inputs['x_thisBranch'].reshape(B, CIN, NJ).astype(bf16)
    xo_full = inputs['x_otherBranch'].reshape(B, CIN, NJ).astype(bf16)
    wtT = np.ascontiguousarray(inputs['w_theta'].T.astype(bf16))
    wpT = np.ascontiguousarray(inputs['w_phi'].T.astype(bf16))
    w_zg = (inputs['w_z'].astype(np.float64) @ inputs['w_g'].astype(np.float64))
    wzgT = np.ascontiguousarray(w_zg.T.astype(np.float32).astype(bf16))
    consts = np.zeros((CI, 8), np.float32)
    consts[:, 0] = inputs['b_phi']
    consts[:, 1] = inputs['bn_gamma']
    consts[:, 2] = inputs['bn_beta']
    consts[:, 3] = SHIFT
    consts[:, 4] = BN_EPS
    consts[:, 5] = 1.0 / NTOT
    in_maps = []
    for c in range(8):
        b, h = c // 2, c % 2
        in_maps.append({
            "xtb": np.ascontiguousarray(xt_full[b]),
            "xob": np.ascontiguousarray(xo_full[b][:, h * NI:(h + 1) * NI]),
            "wtT": wtT, "wpT": wpT, "wzgT": wzgT, "consts": consts,
        })
    return in_maps


def kernel(**inputs):
    from concourse.bass_utils import run_bass_kernel_spmd
    if "nc" not in _CACHE:
        _CACHE["nc"] = _build()
    nc = _CACHE["nc"]
    in_maps = _prep_in_maps(inputs)
    res = run_bass_kernel_spmd(nc, in_maps, list(range(8)))
    out = np.empty((B, CI, NJ), np.float32)
    for c in range(8):
        b, h = c // 2, c % 2
        out[b][:, h * NI:(h + 1) * NI] = res.results[c]["z"]
    return out.reshape(B, CI, H, W)


if __name__ == "__main__":
    inputs = np.load('/tmp/ref_inputs.npy', allow_pickle=True).item()
    ref = np.load('/tmp/ref_output.npy')
    got = kernel(**inputs)
    err = np.abs(got - ref)
    denom = np.abs(ref).max()
    print(f"abs max err: {err.max():.4e}  (ref absmax {denom:.3f})")
    print(f"Relative error: {err.max() / denom:.4e}")
